# revision 2
# baseline (speedup 1.0000x reference)
"""BinaryLinear kernel for Trainium2, data-parallel over 8 NeuronCores.

Computes y = x @ (sign(W) * scale).T + b where
  sign(w) = +1 if w >= 0 else -1
  scale_o = max(mean_i |W[o,i]|, 1e-6)           (per output row)

Strategy
--------
- Shard batch (32768) across 8 cores -> 4096 rows/core; replicate W, b.
- Host passes per core: xT = x_shard.T cast to bf16 ([1024 in, 4096 nb]),
  WT = W.T as f32 ([1024 in, 1024 out]), b f32.
- Device (per core):
    S^T[i,o]  = 2*(WT[i,o] >= 0) - 1          exact +-1 in bf16
    sums[o]   = sum_i |WT[i,o]|               PE matmul with ones vector
    scale[o]  = max(sums/1024, 1e-6)          f32, per-partition layout
    yT[o,nb]  = scale[o] * sum_i S^T[i,o]*xT[i,nb] + b[o]
  Matmuls are bf16 (weights exactly +-1 so no quantization there),
  accumulated in f32 PSUM; scale+bias applied in f32 by one DVE
  tensor_scalar per tile (per-partition scalars since o is the
  partition dim of the output).
- Host transposes yT back and concatenates shards.
"""

import os
import sys
import types

for _p in ("/opt/trn_rl_repo",):
    if _p not in sys.path and os.path.isdir(_p):
        sys.path.append(_p)

import numpy as np
import ml_dtypes

import concourse.bacc as bacc
import concourse.mybir as mybir
from concourse import tile
from concourse.bass_utils import run_bass_kernel_spmd

N_CORES = 8
BATCH = 32768
SHARD = BATCH // N_CORES          # 4096 rows per core
IN = 1024
OUT = 1024
EPS = 1e-6
P = 128                           # SBUF partitions
KC = IN // P                      # 8 contraction chunks
OC = OUT // P                     # 8 output-feature chunks
NB = 512                          # moving free-dim per matmul
NBC = SHARD // NB                 # 8 batch tiles per core

F32 = mybir.dt.float32
BF16 = mybir.dt.bfloat16
Alu = mybir.AluOpType


def _install_trace_shim():
    """antenv.axon_hooks is absent in this image; recreate it so
    run_bass_kernel_spmd(trace=True) can capture NTFF profiles."""
    try:
        import antenv.axon_hooks  # noqa: F401
        return
    except ImportError:
        pass
    try:
        import trn_agent_boot.trn_boot as tb
        hooks = types.ModuleType("antenv.axon_hooks")
        hooks._hook = tb._ntff_profile_via_ctypes("/opt/axon/libaxon_pjrt.so")
        hooks.get_axon_ntff_profile_hook = lambda: hooks._hook
        hooks.set_axon_ntff_profile_hook = lambda h: setattr(hooks, "_hook", h)
        sys.modules["antenv.axon_hooks"] = hooks
        import concourse.bass_utils as bass_utils
        bass_utils.upload_artifacts = lambda tmpdir: f"file://{tmpdir}"
    except Exception:
        pass


def build_program():
    nc = bacc.Bacc("TRN2", target_bir_lowering=False, debug=False,
                   num_devices=N_CORES)

    xt_d = nc.dram_tensor("xt", [IN, SHARD], BF16, kind="ExternalInput")
    wt_d = nc.dram_tensor("wt", [IN, OUT], F32, kind="ExternalInput")
    b_d = nc.dram_tensor("b", [OUT], F32, kind="ExternalInput")
    yt_d = nc.dram_tensor("yt", [OUT, SHARD], F32, kind="ExternalOutput")

    with tile.TileContext(nc) as tc:
        with (
            tc.tile_pool(name="xt_pool", bufs=1) as xt_pool,
            tc.tile_pool(name="wt_pool", bufs=1) as wt_pool,
            tc.tile_pool(name="st_pool", bufs=1) as st_pool,
            tc.tile_pool(name="misc", bufs=1) as misc,
            tc.tile_pool(name="ps", bufs=8, space="PSUM") as ps_pool,
            tc.tile_pool(name="yo_pool", bufs=4) as yo_pool,
        ):
            # ---- input DMAs --------------------------------------------
            wt = []
            for i in range(KC):
                w = wt_pool.tile([P, OUT], F32, tag=f"wt{i}", name=f"wt{i}")
                nc.sync.dma_start(w[:], wt_d.ap()[i * P:(i + 1) * P, :])
                wt.append(w)
            bcol = misc.tile([P, OC], F32, tag="bcol", name="bcol")
            nc.sync.dma_start(bcol[:], b_d.ap().rearrange("(c p) -> p c", p=P))
            xt = []
            for i in range(KC):
                t = xt_pool.tile([P, SHARD], BF16, tag=f"xt{i}", name=f"xt{i}")
                nc.sync.dma_start(t[:], xt_d.ap()[i * P:(i + 1) * P, :])
                xt.append(t)

            # ---- prep: signs and |W| in bf16 ---------------------------
            st, absw = [], []
            for i in range(KC):
                s = st_pool.tile([P, OUT], BF16, tag=f"st{i}", name=f"st{i}")
                nc.vector.tensor_scalar(s[:], wt[i][:], 0.0, None, Alu.is_ge)
                nc.vector.tensor_scalar(s[:], s[:], 2.0, -1.0, Alu.mult, Alu.add)
                a = st_pool.tile([P, OUT], BF16, tag=f"absw{i}", name=f"absw{i}")
                nc.scalar.activation(a[:], wt[i][:], mybir.ActivationFunctionType.Abs)
                st.append(s)
                absw.append(a)

            ones = misc.tile([P, 1], BF16, tag="ones", name="ones")
            nc.vector.memset(ones[:], 1.0)

            # ---- scale[o] = max(mean_i |W|, eps), per-partition layout -
            sp = ps_pool.tile([P, NB], F32, tag="ps", name="sp")
            for c in range(OC):
                for i in range(KC):
                    nc.tensor.matmul(
                        sp[:, c:c + 1],
                        absw[i][:, c * P:(c + 1) * P],
                        ones[:],
                        start=(i == 0), stop=(i == KC - 1),
                    )
            scale = misc.tile([P, OC], F32, tag="scale", name="scale")
            nc.vector.tensor_scalar(scale[:], sp[:, 0:OC], 1.0 / IN, EPS,
                                    Alu.mult, Alu.max)

            # ---- main: yT[o, nb] with scale+bias epilogue --------------
            for c in range(OC):
                yps = []
                for n in range(NBC):
                    yp = ps_pool.tile([P, NB], F32, tag="ps", name=f"yp{c}_{n}")
                    yps.append(yp)
                for i in range(KC):
                    lhsT = st[i][:, c * P:(c + 1) * P]
                    for n in range(NBC):
                        nc.tensor.matmul(
                            yps[n][:],
                            lhsT,
                            xt[i][:, n * NB:(n + 1) * NB],
                            start=(i == 0), stop=(i == KC - 1),
                        )
                for n in range(NBC):
                    yo = yo_pool.tile([P, NB], F32, tag="yo", name=f"yo{c}_{n}")
                    nc.vector.tensor_scalar(yo[:], yps[n][:],
                                            scale[:, c:c + 1], bcol[:, c:c + 1],
                                            Alu.mult, Alu.add)
                    nc.sync.dma_start(
                        yt_d.ap()[c * P:(c + 1) * P, n * NB:(n + 1) * NB],
                        yo[:])

    nc.compile()
    return nc


_NC = None


def _get_program():
    global _NC
    if _NC is None:
        _NC = build_program()
    return _NC


def kernel(x: np.ndarray, W: np.ndarray, b: np.ndarray) -> np.ndarray:
    assert x.shape == (BATCH, IN) and W.shape == (OUT, IN) and b.shape == (OUT,)
    nc = _get_program()

    WT = np.ascontiguousarray(W.T.astype(np.float32, copy=False))
    b32 = np.ascontiguousarray(b.astype(np.float32, copy=False))
    in_maps = []
    for c in range(N_CORES):
        shard = x[c * SHARD:(c + 1) * SHARD]
        xtc = shard.T.astype(ml_dtypes.bfloat16)
        in_maps.append({"xt": xtc, "wt": WT, "b": b32})

    trace = bool(int(os.environ.get("BINLIN_TRACE", "0")))
    if trace:
        _install_trace_shim()
    res = run_bass_kernel_spmd(nc, in_maps, core_ids=list(range(N_CORES)),
                               trace=trace)
    if trace and res.exec_time_ns is not None:
        print(f"HW exec time: {res.exec_time_ns} ns", flush=True)

    y = np.empty((BATCH, OUT), dtype=np.float32)
    for c in range(N_CORES):
        y[c * SHARD:(c + 1) * SHARD] = res.results[c]["yt"].T
    return y


# revision 4
# speedup vs baseline: 1.0822x; 1.0822x over previous
"""BinaryLinear kernel for Trainium2, data-parallel over 8 NeuronCores.

Computes y = x @ (sign(W) * scale).T + b where
  sign(w) = +1 if w >= 0 else -1
  scale_o = max(mean_i |W[o,i]|, 1e-6)           (per output row)

Strategy
--------
- Shard batch (32768) across 8 cores -> 4096 rows/core; replicate W, b.
- Host passes per core (bf16 cast is exact for the +-1 weights and costs
  <0.2% relative error on x, well inside fp32-reference tolerance):
    xt = x_shard.T  bf16 [1024 in, 4096 nb]
    wt = W.T        bf16 [1024 in, 1024 out]   (lhsT source for matmuls)
    wn = W          bf16 [1024 out, 1024 in]   (scale reduction source)
    b  = f32 [1024]
- Device (per core):
    S^T[i,o]  = 2*(wt[i,o] >= 0) - 1          exact +-1 in bf16 (DVE)
    mean[o]   = sum_i |wn[o,i]| / 1024        ACT Abs with accum_out
    scale[o]  = max(mean, 1e-6)               DVE, f32, per-partition
    yT[o,nb]  = scale[o]*sum_i S^T[i,o]*xt[i,nb] + b[o]
  Main loop is batch-block-outer so the first matmuls only need the
  first 2 MB of xt; bf16 matmuls accumulate f32 in PSUM; one DVE
  tensor_scalar per [128,512] tile applies scale+bias (per-partition
  scalars since o is the partition dim of yT).
- Host transposes yT back and concatenates shards.
"""

import os
import sys
import types

for _p in ("/opt/trn_rl_repo",):
    if _p not in sys.path and os.path.isdir(_p):
        sys.path.append(_p)

import numpy as np
import ml_dtypes

import concourse.bacc as bacc
import concourse.mybir as mybir
from concourse import tile
from concourse.bass_utils import run_bass_kernel_spmd

N_CORES = 8
BATCH = 32768
SHARD = BATCH // N_CORES          # 4096 rows per core
IN = 1024
OUT = 1024
EPS = 1e-6
P = 128                           # SBUF partitions
KC = IN // P                      # 8 contraction chunks
OC = OUT // P                     # 8 output-feature chunks
NB = 512                          # moving free-dim per matmul
NBC = SHARD // NB                 # 8 batch blocks per core
NP = NBC // 2                     # xt DMA'd in pairs of batch blocks

F32 = mybir.dt.float32
BF16 = mybir.dt.bfloat16
Alu = mybir.AluOpType
Act = mybir.ActivationFunctionType


def _install_trace_shim():
    """antenv.axon_hooks is absent in this image; recreate it so
    run_bass_kernel_spmd(trace=True) can capture NTFF profiles."""
    try:
        import antenv.axon_hooks  # noqa: F401
        return
    except ImportError:
        pass
    try:
        import trn_agent_boot.trn_boot as tb
        hooks = types.ModuleType("antenv.axon_hooks")
        hooks._hook = tb._ntff_profile_via_ctypes("/opt/axon/libaxon_pjrt.so")
        hooks.get_axon_ntff_profile_hook = lambda: hooks._hook
        hooks.set_axon_ntff_profile_hook = lambda h: setattr(hooks, "_hook", h)
        sys.modules["antenv.axon_hooks"] = hooks
        import concourse.bass_utils as bass_utils
        bass_utils.upload_artifacts = lambda tmpdir: f"file://{tmpdir}"
    except Exception:
        pass


def build_program():
    nc = bacc.Bacc("TRN2", target_bir_lowering=False, debug=False,
                   num_devices=N_CORES)

    xt_d = nc.dram_tensor("xt", [IN, SHARD], BF16, kind="ExternalInput")
    wt_d = nc.dram_tensor("wt", [IN, OUT], BF16, kind="ExternalInput")
    wn_d = nc.dram_tensor("wn", [OUT, IN], BF16, kind="ExternalInput")
    b_d = nc.dram_tensor("b", [OUT], F32, kind="ExternalInput")
    yt_d = nc.dram_tensor("yt", [OUT, SHARD], F32, kind="ExternalOutput")

    with tile.TileContext(nc) as tc:
        with (
            tc.tile_pool(name="xtb_pool", bufs=1) as xtb_pool,
            tc.tile_pool(name="w_pool", bufs=1) as w_pool,
            tc.tile_pool(name="misc", bufs=1) as misc,
            tc.tile_pool(name="scr", bufs=2) as scr,
            tc.tile_pool(name="ps", bufs=8, space="PSUM") as ps_pool,
            tc.tile_pool(name="yo_pool", bufs=4) as yo_pool,
        ):
            # ---- weight loads first (small, unblock sign prep) ---------
            wt = []
            for i in range(KC):
                w = w_pool.tile([P, OUT], BF16, tag=f"wt{i}", name=f"wt{i}")
                nc.sync.dma_start(w[:], wt_d.ap()[i * P:(i + 1) * P, :])
                wt.append(w)
            bcol = misc.tile([P, OC], F32, tag="bcol", name="bcol")
            nc.sync.dma_start(bcol[:], b_d.ap().rearrange("(c p) -> p c", p=P))

            # ---- first batch-block-pair of xt, then wn, then the rest --
            xtb = [[None] * NP for _ in range(KC)]

            def load_xtb(i, pr):
                t = xtb_pool.tile([P, 2 * NB], BF16, tag=f"xtb{i}_{pr}",
                                  name=f"xtb{i}_{pr}")
                nc.sync.dma_start(
                    t[:], xt_d.ap()[i * P:(i + 1) * P,
                                    pr * 2 * NB:(pr + 1) * 2 * NB])
                xtb[i][pr] = t

            for i in range(KC):
                load_xtb(i, 0)
            wn = []
            for c in range(OC):
                w = w_pool.tile([P, IN], BF16, tag=f"wn{c}", name=f"wn{c}")
                nc.sync.dma_start(w[:], wn_d.ap()[c * P:(c + 1) * P, :])
                wn.append(w)
            for pr in range(1, NP):
                for i in range(KC):
                    load_xtb(i, pr)

            # ---- sign prep (DVE): S^T = 2*(wt>=0)-1, exact bf16 --------
            st = []
            for i in range(KC):
                s = w_pool.tile([P, OUT], BF16, tag=f"st{i}", name=f"st{i}")
                nc.vector.tensor_scalar(s[:], wt[i][:], 0.0, None, Alu.is_ge)
                nc.vector.tensor_scalar(s[:], s[:], 2.0, -1.0, Alu.mult, Alu.add)
                st.append(s)

            # ---- scale (ACT): mean_i |W[o,:]| via accum_out ------------
            sums = misc.tile([P, OC], F32, tag="sums", name="sums")
            for c in range(OC):
                ascr = scr.tile([P, IN], BF16, tag="ascr", name=f"ascr{c}")
                nc.scalar.activation(ascr[:], wn[c][:], Act.Abs,
                                     scale=1.0 / IN,
                                     accum_out=sums[:, c:c + 1])
            scale = misc.tile([P, OC], F32, tag="scale", name="scale")
            nc.vector.tensor_scalar(scale[:], sums[:], EPS, None, Alu.max)

            # ---- main loop: batch-block outer ---------------------------
            for n in range(NBC):
                pr, half = divmod(n, 2)
                for c in range(OC):
                    yp = ps_pool.tile([P, NB], F32, tag="ps", name=f"yp{n}_{c}")
                    for i in range(KC):
                        nc.tensor.matmul(
                            yp[:],
                            st[i][:, c * P:(c + 1) * P],
                            xtb[i][pr][:, half * NB:(half + 1) * NB],
                            start=(i == 0), stop=(i == KC - 1),
                        )
                    yo = yo_pool.tile([P, NB], F32, tag="yo", name=f"yo{n}_{c}")
                    nc.vector.tensor_scalar(yo[:], yp[:],
                                            scale[:, c:c + 1], bcol[:, c:c + 1],
                                            Alu.mult, Alu.add)
                    nc.sync.dma_start(
                        yt_d.ap()[c * P:(c + 1) * P, n * NB:(n + 1) * NB],
                        yo[:])

    nc.compile()
    return nc


_NC = None


def _get_program():
    global _NC
    if _NC is None:
        _NC = build_program()
    return _NC


def kernel(x: np.ndarray, W: np.ndarray, b: np.ndarray) -> np.ndarray:
    assert x.shape == (BATCH, IN) and W.shape == (OUT, IN) and b.shape == (OUT,)
    nc = _get_program()

    Wf = np.asarray(W, dtype=np.float32)
    WT = Wf.T.astype(ml_dtypes.bfloat16)
    WN = Wf.astype(ml_dtypes.bfloat16)
    b32 = np.ascontiguousarray(np.asarray(b, dtype=np.float32))
    in_maps = []
    for c in range(N_CORES):
        shard = x[c * SHARD:(c + 1) * SHARD]
        xtc = shard.T.astype(ml_dtypes.bfloat16)
        in_maps.append({"xt": xtc, "wt": WT, "wn": WN, "b": b32})

    trace = bool(int(os.environ.get("BINLIN_TRACE", "0")))
    if trace:
        _install_trace_shim()
    res = run_bass_kernel_spmd(nc, in_maps, core_ids=list(range(N_CORES)),
                               trace=trace)
    if trace and res.exec_time_ns is not None:
        print(f"HW exec time: {res.exec_time_ns} ns", flush=True)

    y = np.empty((BATCH, OUT), dtype=np.float32)
    for c in range(N_CORES):
        y[c * SHARD:(c + 1) * SHARD] = res.results[c]["yt"].T
    return y
